# revision 1
# baseline (speedup 1.0000x reference)
"""DirGCNConv (weighted gather + segment_sum + linear) on 8 Trainium2 NeuronCores.

Computation (reference):
    dst, src = edge_index
    agg[d] = sum_{e: dst_e == d} edge_weight[e] * x[src_e]     # [N, D]
    out = agg @ W.T + b

Strategy (dst-sharded, no collectives):
  - Host: shard edges by dst node range (N/8 = 12500 nodes per core). Fixed
    dst tiles of 256 nodes (T=49 per core). Within a tile, edges are grouped
    by src bank (4 banks of 25000 rows so bank-local ids fit int16 for the
    dma_gather ucode), sorted by src for HBM row locality, and padded per
    (tile, bank) to a multiple of 128 edges; the padded count is the max
    over cores so all 8 cores share one program (SPMD).
  - Device, per group of GT=2 tiles:
      * 4 dma_gather ucode calls (InstDMAGatherAnt, mlp library) gather all
        the group's edges' source rows from the bf16 bank tables in HBM into
        an SBUF buffer G[128, nblk, 128]: stream slot i -> partition i%128,
        block i//128 (matches the 128-edge matmul chunk layout).
        One instruction per (group, bank) costs ~1us + 0.34ns/row of Pool
        time vs ~1.1us per 128 rows for the old per-chunk indirect DMA.
      * per 128-edge chunk j: DVE tensor_scalar builds the weighted one-hot
        S[e, dstloc] = (iota == dl_e) * w_e  [128x256 bf16, 4x perf mode],
        TensorE accumulates PSUM agg.T[din, dst256] += G_j.T @ S_j.
      * per tile: PSUM -> SBUF (ACT), out.T = W @ agg.T (fp32 matmul with
        stationary W.T), + bias via ACT per-partition bias, DMA out.
  - Host: out rows of core c = outT[:, :12500].T (tile t covers local nodes
    t*256..t*256+255, so column == local node id).
"""

import numpy as np
import ml_dtypes

bf16 = ml_dtypes.bfloat16

# problem constants (hardcoded per harness contract)
N_NODES = 100000
N_EDGES = 3200000
D = 128
NCORES = 8

# design constants
NLOC = N_NODES // NCORES      # 12500 dst nodes per core
TILE_W = 256                  # dst columns per tile
T_TILES = (NLOC + TILE_W - 1) // TILE_W   # 49
CHUNK = 128                   # edges per matmul chunk
NBANKS = 4
BROWS = N_NODES // NBANKS     # 25000 rows per src bank (int16-safe)
GT = 2                        # tiles per gather group


def _host_prep(x, edge_index, edge_weight):
    """Vectorized host prep. Returns per-core streams + static block counts."""
    dst = np.asarray(edge_index[0], dtype=np.int64)
    src = np.asarray(edge_index[1], dtype=np.int64)
    w = np.asarray(edge_weight, dtype=np.float32)

    order = np.argsort(dst, kind="stable")
    dst_s, src_s, w_s = dst[order], src[order], w[order]
    bounds = np.searchsorted(dst_s, np.arange(NCORES + 1) * NLOC)

    cores = []
    counts_all = np.zeros((NCORES, T_TILES, NBANKS), np.int64)
    for c in range(NCORES):
        lo, hi = int(bounds[c]), int(bounds[c + 1])
        d_l = dst_s[lo:hi] - c * NLOC
        s_l = src_s[lo:hi]
        w_l = w_s[lo:hi]
        tile = d_l // TILE_W
        bank = s_l // BROWS
        grp = tile // GT
        # sort into segment-iteration order: (group, bank, tile), src minor
        o = np.lexsort((s_l, tile, bank, grp))
        tile, bank = tile[o], bank[o]
        cores.append((d_l[o], s_l[o], w_l[o], tile, bank))
        np.add.at(counts_all[c], (tile, bank), 1)

    # static padded counts per (tile, bank): max over cores, ceil to CHUNK
    P = ((counts_all.max(axis=0) + CHUNK - 1) // CHUNK) * CHUNK  # [T, NBANKS]
    nblk = P // CHUNK

    # segment-iteration order (group, bank, tile) -> slot offsets
    n_groups = (T_TILES + GT - 1) // GT
    seg_order = []  # (t, b) in iteration order
    for g in range(n_groups):
        for b in range(NBANKS):
            for t in range(g * GT, min((g + 1) * GT, T_TILES)):
                seg_order.append((t, b))
    seg_sizes = np.array([P[t, b] for t, b in seg_order], np.int64)
    seg_starts = np.concatenate([[0], np.cumsum(seg_sizes)])
    tot_slots = int(seg_starts[-1])
    totblk = tot_slots // CHUNK
    # map (t, b) -> slot start
    seg_start_tb = np.zeros((T_TILES, NBANKS), np.int64)
    for i, (t, b) in enumerate(seg_order):
        seg_start_tb[t, b] = seg_starts[i]

    # per-group metadata for the program builder
    groups = []
    for g in range(n_groups):
        tiles_g = list(range(g * GT, min((g + 1) * GT, T_TILES)))
        g_slot0 = int(seg_start_tb[tiles_g[0], 0])
        g_blk0 = g_slot0 // CHUNK
        bank_segs = []  # (local block offset, num blocks) per bank
        for b in range(NBANKS):
            s0 = int(seg_start_tb[tiles_g[0], b])
            ln = int(sum(P[t, b] for t in tiles_g))
            bank_segs.append(((s0 - g_slot0) // CHUNK, ln // CHUNK))
        tile_blocks = []  # per tile: list of local block indices (group-rel)
        for t in tiles_g:
            blks = []
            for b in range(NBANKS):
                s0 = int(seg_start_tb[t, b])
                blks.extend(range((s0 - g_slot0) // CHUNK,
                                  (s0 - g_slot0) // CHUNK + int(nblk[t, b])))
            tile_blocks.append((t, blks))
        nblk_g = int(sum(P[t, b] for t in tiles_g for b in range(NBANKS))) // CHUNK
        groups.append({
            "blk0": g_blk0, "nblk": nblk_g,
            "bank_segs": bank_segs, "tile_blocks": tile_blocks,
        })

    # per-core streams
    per_core = []
    for c in range(NCORES):
        d_l, s_l, w_l, tile, bank = cores[c]
        ne = len(d_l)
        # edges are sorted in segment-iteration order; compute positions
        cnt_iter = np.array(
            [counts_all[c, t, b] for t, b in seg_order], np.int64)
        first = np.concatenate([[0], np.cumsum(cnt_iter)])[:-1]
        pos = (np.repeat(seg_starts[:-1], cnt_iter)
               + np.arange(ne) - np.repeat(first, cnt_iter))

        idx_stream = np.zeros(tot_slots, np.int16)
        wq_stream = np.zeros(tot_slots, np.float32)
        dl_stream = np.zeros(tot_slots, np.float32)
        idx_stream[pos] = (s_l % BROWS).astype(np.int16)
        wq_stream[pos] = w_l
        dl_stream[pos] = (d_l % TILE_W).astype(np.float32)

        idx16 = np.ascontiguousarray(
            np.tile(idx_stream.reshape(-1, 16).T, (8, 1)))   # [128, totblk*8]
        dl_cols = np.ascontiguousarray(
            dl_stream.reshape(totblk, CHUNK).T)               # [128, totblk]
        w_cols = np.ascontiguousarray(
            wq_stream.reshape(totblk, CHUNK).T)               # [128, totblk]
        per_core.append({"idx16": idx16, "dl": dl_cols, "w": w_cols})

    xb = np.asarray(x, np.float32).astype(bf16)  # [N, D]
    banks = [np.ascontiguousarray(xb[b * BROWS:(b + 1) * BROWS])
             for b in range(NBANKS)]

    iota = np.ascontiguousarray(np.broadcast_to(
        np.arange(TILE_W, dtype=np.float32), (128, TILE_W)).astype(bf16))

    return per_core, banks, iota, groups, totblk


def _build_program(groups, totblk):
    import concourse.bass as bass  # noqa: F401
    import concourse.bacc as bacc
    import concourse.mybir as mybir
    import concourse.tile as tile
    from concourse import library_config

    nc = bacc.Bacc("TRN2", target_bir_lowering=False, debug=False,
                   num_devices=NCORES)

    xb_d = [nc.dram_tensor(f"xb{b}", [BROWS, D], mybir.dt.bfloat16,
                           kind="ExternalInput") for b in range(NBANKS)]
    idx_d = nc.dram_tensor("idx16", [128, totblk * 8], mybir.dt.int16,
                           kind="ExternalInput")
    dl_d = nc.dram_tensor("dl", [128, totblk], mybir.dt.float32,
                          kind="ExternalInput")
    w_d = nc.dram_tensor("w", [128, totblk], mybir.dt.float32,
                         kind="ExternalInput")
    iota_d = nc.dram_tensor("iota", [128, TILE_W], mybir.dt.bfloat16,
                            kind="ExternalInput")
    wt_d = nc.dram_tensor("wt", [D, D], mybir.dt.float32, kind="ExternalInput")
    b_d = nc.dram_tensor("b", [D, 1], mybir.dt.float32, kind="ExternalInput")
    out_d = nc.dram_tensor("outT", [D, T_TILES * TILE_W], mybir.dt.float32,
                           kind="ExternalOutput")

    max_nblk = max(g["nblk"] for g in groups)

    with tile.TileContext(nc) as tc:
        with (
            tc.tile_pool(name="const", bufs=1) as constp,
            tc.tile_pool(name="meta", bufs=3) as metap,
            tc.tile_pool(name="gather", bufs=2) as gatherp,
            tc.tile_pool(name="s", bufs=6) as sp,
            tc.tile_pool(name="agg", bufs=2) as aggp,
            tc.tile_pool(name="outp", bufs=2) as outp,
            tc.tile_pool(name="psum", bufs=2, space="PSUM") as psump,
            tc.tile_pool(name="psum2", bufs=2, space="PSUM") as psum2p,
        ):
            nc.gpsimd.load_library(library_config.mlp)

            iota_t = constp.tile([128, TILE_W], mybir.dt.bfloat16)
            wt_t = constp.tile([D, D], mybir.dt.float32)
            b_t = constp.tile([D, 1], mybir.dt.float32)
            nc.sync.dma_start(iota_t[:], iota_d[:])
            nc.sync.dma_start(wt_t[:], wt_d[:])
            nc.sync.dma_start(b_t[:], b_d[:])

            for g in groups:
                blk0, nblk_g = g["blk0"], g["nblk"]
                idx_t = metap.tile([128, max_nblk * 8], mybir.dt.int16,
                                   tag="idx")
                dl_t = metap.tile([128, max_nblk], mybir.dt.float32, tag="dl")
                w_t = metap.tile([128, max_nblk], mybir.dt.float32, tag="w")
                nc.sync.dma_start(idx_t[:, :nblk_g * 8],
                                  idx_d[:, blk0 * 8:(blk0 + nblk_g) * 8])
                nc.sync.dma_start(dl_t[:, :nblk_g],
                                  dl_d[:, blk0:blk0 + nblk_g])
                nc.sync.dma_start(w_t[:, :nblk_g],
                                  w_d[:, blk0:blk0 + nblk_g])

                g_t = gatherp.tile([128, max_nblk, D], mybir.dt.bfloat16,
                                   tag="g")
                # SWDGE descriptor ring caps one gather at 1024 descriptors
                # (ucode-fixed), i.e. 8 blocks of 128 rows.
                MAXG = 8
                for b in range(NBANKS):
                    boff, blen = g["bank_segs"][b]
                    for off in range(0, blen, MAXG):
                        ln = min(MAXG, blen - off)
                        a = boff + off
                        nc.gpsimd.dma_gather(
                            g_t[:, a:a + ln, :],
                            xb_d[b][:],
                            idx_t[:, a * 8:(a + ln) * 8],
                            ln * CHUNK,
                            ln * CHUNK,
                            D,
                        )

                for t, blks in g["tile_blocks"]:
                    assert blks, f"tile {t} has no edge blocks"
                    psum_t = psump.tile([D, TILE_W], mybir.dt.float32,
                                        tag="p1")
                    for k, j in enumerate(blks):
                        s_t = sp.tile([128, TILE_W], mybir.dt.bfloat16,
                                      tag="s")
                        nc.vector.tensor_scalar(
                            s_t[:], iota_t[:], dl_t[:, j:j + 1],
                            w_t[:, j:j + 1],
                            mybir.AluOpType.is_equal, mybir.AluOpType.mult,
                        )
                        nc.tensor.matmul(
                            psum_t[:], g_t[:, j, :], s_t[:],
                            start=(k == 0), stop=(k == len(blks) - 1),
                        )

                    aggT_t = aggp.tile([D, TILE_W], mybir.dt.float32,
                                       tag="agg")
                    nc.scalar.copy(aggT_t[:], psum_t[:])

                    psum2_t = psum2p.tile([D, TILE_W], mybir.dt.float32,
                                          tag="p2")
                    nc.tensor.matmul(psum2_t[:], wt_t[:], aggT_t[:],
                                     start=True, stop=True)

                    out_t = outp.tile([D, TILE_W], mybir.dt.float32, tag="o")
                    nc.scalar.activation(
                        out_t[:], psum2_t[:],
                        mybir.ActivationFunctionType.Identity,
                        bias=b_t[:, 0:1], scale=1.0,
                    )
                    nc.sync.dma_start(
                        out_d[:, t * TILE_W:(t + 1) * TILE_W], out_t[:])

    nc.compile()
    return nc


LAST_RES = None


def kernel(x, edge_index, edge_weight, W, b):
    import os
    from concourse.bass_utils import run_bass_kernel_spmd

    per_core, banks, iota, groups, totblk = _host_prep(
        x, edge_index, edge_weight)

    nc = _build_program(groups, totblk)

    WT = np.ascontiguousarray(np.asarray(W, np.float32).T)  # [din, dout]
    bcol = np.ascontiguousarray(np.asarray(b, np.float32).reshape(D, 1))

    in_maps = []
    for c in range(NCORES):
        p = per_core[c]
        m = {f"xb{i}": banks[i] for i in range(NBANKS)}
        m.update({
            "idx16": p["idx16"], "dl": p["dl"], "w": p["w"],
            "iota": iota, "wt": WT, "b": bcol,
        })
        in_maps.append(m)

    res = run_bass_kernel_spmd(
        nc, in_maps, core_ids=list(range(NCORES)),
        trace=bool(int(os.environ.get("KERNEL_TRACE", "0"))),
    )
    global LAST_RES
    LAST_RES = res

    out = np.empty((N_NODES, D), np.float32)
    for c in range(NCORES):
        outT = res.results[c]["outT"]  # [D, T*TILE_W]
        out[c * NLOC:(c + 1) * NLOC] = outT[:, :NLOC].T
    return out


if __name__ == "__main__":
    # smoke test with random data (self-contained)
    rng = np.random.default_rng(0)
    x = rng.standard_normal((N_NODES, D)).astype(np.float32)
    ei = rng.integers(0, N_NODES, size=(2, N_EDGES)).astype(np.int64)
    ew = rng.random(N_EDGES).astype(np.float32)
    W = (rng.standard_normal((D, D)) / np.sqrt(D)).astype(np.float32)
    b = (rng.standard_normal(D) * 0.01).astype(np.float32)
    out = kernel(x, ei, ew, W, b)
    print("out", out.shape, out.dtype)



# revision 2
# speedup vs baseline: 1.5367x; 1.5367x over previous
"""DirGCNConv (weighted gather + segment_sum + linear) on 8 Trainium2 NeuronCores.

Computation (reference):
    dst, src = edge_index
    agg[d] = sum_{e: dst_e == d} edge_weight[e] * x[src_e]     # [N, D]
    out = agg @ W.T + b

Strategy (dst-sharded, no collectives):
  - Host: shard edges by dst node range (N/8 = 12500 nodes per core). Fixed
    dst tiles of 128 nodes (T=98 per core). Within a tile, edges are grouped
    by src bank (4 banks of 25000 rows so bank-local ids fit int16 for the
    dma_gather ucode), sorted by src for HBM row locality, and padded per
    (tile, bank) to a multiple of 128 edges; the padded count is the max
    over cores so all 8 cores share one program (SPMD).
  - Device, per group of GT=4 tiles (one 512-wide output supertile):
      * dma_gather ucode calls (InstDMAGatherAnt, mlp library) gather the
        group's edges' source rows from the bf16 bank tables in HBM into
        SBUF G[128, nblk, 128]: stream slot i -> partition i%128, block
        i//128 (matches the 128-edge matmul chunk layout). Calls for bank b
        use SWDGE queue b, so descriptor generation runs on all 4 Q7 core
        pairs concurrently (the ucode gates on cpu_id/2 == queue_num; with
        one queue it is a single-pair serial bottleneck at ~8.3 ns/row).
      * per 128-edge chunk j: DVE tensor_scalar builds the weighted one-hot
        S[e, dstloc] = (iota == dl_e) * w_e  [128x128 bf16],
        TensorE accumulates PSUM agg.T[din, dst128] += G_j.T @ S_j.
      * per tile: PSUM -> SBUF staging agg4[128, 512] (ACT copy into the
        tile's 128-column slice); per group: out.T = W @ agg4 (one fp32
        matmul), + bias via ACT per-partition bias, one 256 KB DMA out.
  - Host: out rows of core c = outT[:, :12500].T (tile t covers local nodes
    t*128..t*128+127, so column == local node id).
"""

import numpy as np
import ml_dtypes

bf16 = ml_dtypes.bfloat16

# problem constants (hardcoded per harness contract)
N_NODES = 100000
N_EDGES = 3200000
D = 128
NCORES = 8

# design constants
NLOC = N_NODES // NCORES      # 12500 dst nodes per core
TILE_W = 128                  # dst columns per tile
T_TILES = (NLOC + TILE_W - 1) // TILE_W   # 98
CHUNK = 128                   # edges per matmul chunk
NBANKS = 4
BROWS = N_NODES // NBANKS     # 25000 rows per src bank (int16-safe)
GT = 4                        # tiles per gather group / output supertile
NQUEUES = 4                   # SWDGE descriptor queues (Q7 core pairs)


def _host_prep(x, edge_index, edge_weight):
    """Vectorized host prep. Returns per-core streams + static block counts."""
    dst = np.asarray(edge_index[0], dtype=np.int64)
    src = np.asarray(edge_index[1], dtype=np.int64)
    w = np.asarray(edge_weight, dtype=np.float32)

    order = np.argsort(dst, kind="stable")
    dst_s, src_s, w_s = dst[order], src[order], w[order]
    bounds = np.searchsorted(dst_s, np.arange(NCORES + 1) * NLOC)

    cores = []
    counts_all = np.zeros((NCORES, T_TILES, NBANKS), np.int64)
    for c in range(NCORES):
        lo, hi = int(bounds[c]), int(bounds[c + 1])
        d_l = dst_s[lo:hi] - c * NLOC
        s_l = src_s[lo:hi]
        w_l = w_s[lo:hi]
        tile = d_l // TILE_W
        bank = s_l // BROWS
        grp = tile // GT
        # sort into segment-iteration order: (group, bank, tile), src minor
        o = np.lexsort((s_l, tile, bank, grp))
        tile, bank = tile[o], bank[o]
        cores.append((d_l[o], s_l[o], w_l[o], tile, bank))
        np.add.at(counts_all[c], (tile, bank), 1)

    # static padded counts per (tile, bank): max over cores, ceil to CHUNK
    P = ((counts_all.max(axis=0) + CHUNK - 1) // CHUNK) * CHUNK  # [T, NBANKS]
    nblk = P // CHUNK

    # segment-iteration order (group, bank, tile) -> slot offsets
    n_groups = (T_TILES + GT - 1) // GT
    seg_order = []  # (t, b) in iteration order
    for g in range(n_groups):
        for b in range(NBANKS):
            for t in range(g * GT, min((g + 1) * GT, T_TILES)):
                seg_order.append((t, b))
    seg_sizes = np.array([P[t, b] for t, b in seg_order], np.int64)
    seg_starts = np.concatenate([[0], np.cumsum(seg_sizes)])
    tot_slots = int(seg_starts[-1])
    totblk = tot_slots // CHUNK
    # map (t, b) -> slot start
    seg_start_tb = np.zeros((T_TILES, NBANKS), np.int64)
    for i, (t, b) in enumerate(seg_order):
        seg_start_tb[t, b] = seg_starts[i]

    # per-group metadata for the program builder
    groups = []
    for g in range(n_groups):
        tiles_g = list(range(g * GT, min((g + 1) * GT, T_TILES)))
        g_slot0 = int(seg_start_tb[tiles_g[0], 0])
        g_blk0 = g_slot0 // CHUNK
        bank_segs = []  # (local block offset, num blocks) per bank
        for b in range(NBANKS):
            s0 = int(seg_start_tb[tiles_g[0], b])
            ln = int(sum(P[t, b] for t in tiles_g))
            bank_segs.append(((s0 - g_slot0) // CHUNK, ln // CHUNK))
        tile_blocks = []  # per tile: list of local block indices (group-rel)
        for t in tiles_g:
            blks = []
            for b in range(NBANKS):
                s0 = int(seg_start_tb[t, b])
                blks.extend(range((s0 - g_slot0) // CHUNK,
                                  (s0 - g_slot0) // CHUNK + int(nblk[t, b])))
            tile_blocks.append((t, blks))
        nblk_g = int(sum(P[t, b] for t in tiles_g for b in range(NBANKS))) // CHUNK
        groups.append({
            "blk0": g_blk0, "nblk": nblk_g,
            "bank_segs": bank_segs, "tile_blocks": tile_blocks,
        })

    # per-core streams
    per_core = []
    for c in range(NCORES):
        d_l, s_l, w_l, tile, bank = cores[c]
        ne = len(d_l)
        # edges are sorted in segment-iteration order; compute positions
        cnt_iter = np.array(
            [counts_all[c, t, b] for t, b in seg_order], np.int64)
        first = np.concatenate([[0], np.cumsum(cnt_iter)])[:-1]
        pos = (np.repeat(seg_starts[:-1], cnt_iter)
               + np.arange(ne) - np.repeat(first, cnt_iter))

        # pad slots repeat the segment's last real src row (HBM row stays
        # hot) and carry w=0 so they contribute nothing.
        idx_stream = np.zeros(tot_slots, np.int16)
        wq_stream = np.zeros(tot_slots, np.float32)
        dl_stream = np.zeros(tot_slots, np.float32)
        idx_stream[pos] = (s_l % BROWS).astype(np.int16)
        # forward-fill pad slots within each segment from the last real edge
        valid = np.zeros(tot_slots, bool)
        valid[pos] = True
        vi = np.where(valid, np.arange(tot_slots), 0)
        np.maximum.accumulate(vi, out=vi)
        idx_stream = idx_stream[vi]
        wq_stream[pos] = w_l
        dl_stream[pos] = (d_l % TILE_W).astype(np.float32)

        idx16 = np.ascontiguousarray(
            np.tile(idx_stream.reshape(-1, 16).T, (8, 1)))   # [128, totblk*8]
        dl_cols = np.ascontiguousarray(
            dl_stream.reshape(totblk, CHUNK).T)               # [128, totblk]
        w_cols = np.ascontiguousarray(
            wq_stream.reshape(totblk, CHUNK).T)               # [128, totblk]
        per_core.append({"idx16": idx16, "dl": dl_cols, "w": w_cols})

    xb = np.asarray(x, np.float32).astype(bf16)  # [N, D]
    banks = [np.ascontiguousarray(xb[b * BROWS:(b + 1) * BROWS])
             for b in range(NBANKS)]

    iota = np.ascontiguousarray(np.broadcast_to(
        np.arange(TILE_W, dtype=np.float32), (128, TILE_W)).astype(bf16))

    return per_core, banks, iota, groups, totblk


def _build_program(groups, totblk):
    import concourse.bass as bass  # noqa: F401
    import concourse.bacc as bacc
    import concourse.mybir as mybir
    import concourse.tile as tile
    from concourse import library_config

    nc = bacc.Bacc("TRN2", target_bir_lowering=False, debug=False,
                   num_devices=NCORES, num_swdge_queues=NQUEUES)

    xb_d = [nc.dram_tensor(f"xb{b}", [BROWS, D], mybir.dt.bfloat16,
                           kind="ExternalInput") for b in range(NBANKS)]
    idx_d = nc.dram_tensor("idx16", [128, totblk * 8], mybir.dt.int16,
                           kind="ExternalInput")
    dl_d = nc.dram_tensor("dl", [128, totblk], mybir.dt.float32,
                          kind="ExternalInput")
    w_d = nc.dram_tensor("w", [128, totblk], mybir.dt.float32,
                         kind="ExternalInput")
    iota_d = nc.dram_tensor("iota", [128, TILE_W], mybir.dt.bfloat16,
                            kind="ExternalInput")
    wt_d = nc.dram_tensor("wt", [D, D], mybir.dt.float32, kind="ExternalInput")
    b_d = nc.dram_tensor("b", [D, 1], mybir.dt.float32, kind="ExternalInput")
    out_d = nc.dram_tensor("outT", [D, T_TILES * TILE_W], mybir.dt.float32,
                           kind="ExternalOutput")

    max_nblk = max(g["nblk"] for g in groups)
    SUP_W = GT * TILE_W  # supertile width (512)

    with tile.TileContext(nc) as tc:
        with (
            tc.tile_pool(name="const", bufs=1) as constp,
            tc.tile_pool(name="meta", bufs=3) as metap,
            tc.tile_pool(name="gather", bufs=3) as gatherp,
            tc.tile_pool(name="s", bufs=6) as sp,
            tc.tile_pool(name="agg", bufs=2) as aggp,
            tc.tile_pool(name="outp", bufs=2) as outp,
            tc.tile_pool(name="psum", bufs=4, space="PSUM") as psump,
            tc.tile_pool(name="psum2", bufs=2, space="PSUM") as psum2p,
        ):
            nc.gpsimd.load_library(library_config.mlp)

            iota_t = constp.tile([128, TILE_W], mybir.dt.bfloat16)
            wt_t = constp.tile([D, D], mybir.dt.float32)
            b_t = constp.tile([D, 1], mybir.dt.float32)
            nc.sync.dma_start(iota_t[:], iota_d[:])
            nc.sync.dma_start(wt_t[:], wt_d[:])
            nc.sync.dma_start(b_t[:], b_d[:])

            for gi, g in enumerate(groups):
                blk0, nblk_g = g["blk0"], g["nblk"]
                ntile_g = len(g["tile_blocks"])
                sup_w = ntile_g * TILE_W
                idx_t = metap.tile([128, max_nblk * 8], mybir.dt.int16,
                                   tag="idx")
                dl_t = metap.tile([128, max_nblk], mybir.dt.float32, tag="dl")
                w_t = metap.tile([128, max_nblk], mybir.dt.float32, tag="w")
                nc.sync.dma_start(idx_t[:, :nblk_g * 8],
                                  idx_d[:, blk0 * 8:(blk0 + nblk_g) * 8])
                nc.sync.dma_start(dl_t[:, :nblk_g],
                                  dl_d[:, blk0:blk0 + nblk_g])
                nc.sync.dma_start(w_t[:, :nblk_g],
                                  w_d[:, blk0:blk0 + nblk_g])

                g_t = gatherp.tile([128, max_nblk, D], mybir.dt.bfloat16,
                                   tag="g")
                # SWDGE descriptor ring caps one gather at 1024 descriptors
                # (ucode-fixed), i.e. 8 blocks of 128 rows. Bank b's calls go
                # to SWDGE queue b: descriptor generation for the 4 banks
                # runs on 4 distinct Q7 core pairs concurrently.
                MAXG = 8
                for b in range(NBANKS):
                    boff, blen = g["bank_segs"][b]
                    for off in range(0, blen, MAXG):
                        ln = min(MAXG, blen - off)
                        a = boff + off
                        nc.gpsimd.dma_gather(
                            g_t[:, a:a + ln, :],
                            xb_d[b][:],
                            idx_t[:, a * 8:(a + ln) * 8],
                            ln * CHUNK,
                            ln * CHUNK,
                            D,
                            queue_num=b,
                        )

                agg4_t = aggp.tile([D, SUP_W], mybir.dt.float32, tag="agg")
                for ti, (t, blks) in enumerate(g["tile_blocks"]):
                    assert blks, f"tile {t} has no edge blocks"
                    psum_t = psump.tile([D, TILE_W], mybir.dt.float32,
                                        tag="p1")
                    for k, j in enumerate(blks):
                        s_t = sp.tile([128, TILE_W], mybir.dt.bfloat16,
                                      tag="s")
                        nc.vector.tensor_scalar(
                            s_t[:], iota_t[:], dl_t[:, j:j + 1],
                            w_t[:, j:j + 1],
                            mybir.AluOpType.is_equal, mybir.AluOpType.mult,
                        )
                        nc.tensor.matmul(
                            psum_t[:], g_t[:, j, :], s_t[:],
                            start=(k == 0), stop=(k == len(blks) - 1),
                        )

                    nc.scalar.copy(
                        agg4_t[:, ti * TILE_W:(ti + 1) * TILE_W], psum_t[:])

                psum2_t = psum2p.tile([D, SUP_W], mybir.dt.float32, tag="p2")
                nc.tensor.matmul(psum2_t[:, :sup_w], wt_t[:],
                                 agg4_t[:, :sup_w], start=True, stop=True)

                out_t = outp.tile([D, SUP_W], mybir.dt.float32, tag="o")
                nc.scalar.activation(
                    out_t[:, :sup_w], psum2_t[:, :sup_w],
                    mybir.ActivationFunctionType.Identity,
                    bias=b_t[:, 0:1], scale=1.0,
                )
                t0 = g["tile_blocks"][0][0]
                nc.sync.dma_start(
                    out_d[:, t0 * TILE_W:t0 * TILE_W + sup_w],
                    out_t[:, :sup_w])

    nc.compile()
    return nc


LAST_RES = None


def kernel(x, edge_index, edge_weight, W, b):
    import os
    from concourse.bass_utils import run_bass_kernel_spmd

    per_core, banks, iota, groups, totblk = _host_prep(
        x, edge_index, edge_weight)

    nc = _build_program(groups, totblk)

    WT = np.ascontiguousarray(np.asarray(W, np.float32).T)  # [din, dout]
    bcol = np.ascontiguousarray(np.asarray(b, np.float32).reshape(D, 1))

    in_maps = []
    for c in range(NCORES):
        p = per_core[c]
        m = {f"xb{i}": banks[i] for i in range(NBANKS)}
        m.update({
            "idx16": p["idx16"], "dl": p["dl"], "w": p["w"],
            "iota": iota, "wt": WT, "b": bcol,
        })
        in_maps.append(m)

    res = run_bass_kernel_spmd(
        nc, in_maps, core_ids=list(range(NCORES)),
        trace=bool(int(os.environ.get("KERNEL_TRACE", "0"))),
    )
    global LAST_RES
    LAST_RES = res

    out = np.empty((N_NODES, D), np.float32)
    for c in range(NCORES):
        outT = res.results[c]["outT"]  # [D, T*TILE_W]
        out[c * NLOC:(c + 1) * NLOC] = outT[:, :NLOC].T
    return out


if __name__ == "__main__":
    # smoke test with random data (self-contained)
    rng = np.random.default_rng(0)
    x = rng.standard_normal((N_NODES, D)).astype(np.float32)
    ei = rng.integers(0, N_NODES, size=(2, N_EDGES)).astype(np.int64)
    ew = rng.random(N_EDGES).astype(np.float32)
    W = (rng.standard_normal((D, D)) / np.sqrt(D)).astype(np.float32)
    b = (rng.standard_normal(D) * 0.01).astype(np.float32)
    out = kernel(x, ei, ew, W, b)
    print("out", out.shape, out.dtype)


# revision 6
# speedup vs baseline: 2.1508x; 1.3996x over previous
"""DirGCNConv (weighted gather + segment_sum + linear) on 8 Trainium2 NeuronCores.

Computation (reference):
    dst, src = edge_index
    agg[d] = sum_{e: dst_e == d} edge_weight[e] * x[src_e]     # [N, D]
    out = agg @ W.T + b

Strategy (dst-sharded, no collectives):
  - Host: shard edges by dst node range (N/8 = 12500 nodes per core). Fixed
    dst tiles of 128 nodes (T=98 per core). Within a tile, edges are grouped
    by src bank (4 banks of 25000 rows so bank-local ids fit int16 for the
    dma_gather ucode), sorted by src for HBM row locality, and padded per
    (tile, bank) to a multiple of 128 edges; the padded count is the max
    over cores so all 8 cores share one program (SPMD).
  - Device, per group of GT=4 tiles (one 512-wide output supertile):
      * dma_gather ucode calls (InstDMAGatherAnt, mlp library) gather the
        group's edges' source rows from the bf16 bank tables in HBM into
        SBUF G[128, nblk, 128]: stream slot i -> partition i%128, block
        i//128 (matches the 128-edge matmul chunk layout). Calls for bank b
        use SWDGE queue b, so descriptor generation runs on all 4 Q7 core
        pairs concurrently (the ucode gates on cpu_id/2 == queue_num; with
        one queue it is a single-pair serial bottleneck at ~8.3 ns/row).
      * per 128-edge chunk j: DVE tensor_scalar builds the weighted one-hot
        S[e, dstloc] = (iota == dl_e) * w_e  [128x128 bf16],
        TensorE accumulates PSUM agg.T[din, dst128] += G_j.T @ S_j.
      * per tile: PSUM -> SBUF staging agg4[128, 512] (ACT copy into the
        tile's 128-column slice); per group: out.T = W @ agg4 (one fp32
        matmul), + bias via ACT per-partition bias, one 256 KB DMA out.
  - Host: out rows of core c = outT[:, :12500].T (tile t covers local nodes
    t*128..t*128+127, so column == local node id).
"""

import numpy as np
import ml_dtypes

bf16 = ml_dtypes.bfloat16

# problem constants (hardcoded per harness contract)
N_NODES = 100000
N_EDGES = 3200000
D = 128
NCORES = 8

# design constants
NLOC = N_NODES // NCORES      # 12500 dst nodes per core
TILE_W = 128                  # dst columns per tile
T_TILES = (NLOC + TILE_W - 1) // TILE_W   # 98
CHUNK = 128                   # edges per matmul chunk
NBANKS = 4
BROWS = N_NODES // NBANKS     # 25000 rows per src bank (int16-safe)
GT = 4                        # tiles per gather group / output supertile
NQUEUES = 4                   # SWDGE descriptor queues (Q7 core pairs)


def _host_prep(x, edge_index, edge_weight):
    """Vectorized host prep. Returns per-core streams + static block counts."""
    dst = np.asarray(edge_index[0], dtype=np.int64)
    src = np.asarray(edge_index[1], dtype=np.int64)
    w = np.asarray(edge_weight, dtype=np.float32)

    order = np.argsort(dst, kind="stable")
    dst_s, src_s, w_s = dst[order], src[order], w[order]
    bounds = np.searchsorted(dst_s, np.arange(NCORES + 1) * NLOC)

    cores = []
    counts_all = np.zeros((NCORES, T_TILES, NBANKS), np.int64)
    for c in range(NCORES):
        lo, hi = int(bounds[c]), int(bounds[c + 1])
        d_l = dst_s[lo:hi] - c * NLOC
        s_l = src_s[lo:hi]
        w_l = w_s[lo:hi]
        tile = d_l // TILE_W
        bank = s_l // BROWS
        grp = tile // GT
        # sort into segment-iteration order: (group, bank, tile), src minor
        o = np.lexsort((s_l, tile, bank, grp))
        tile, bank = tile[o], bank[o]
        cores.append((d_l[o], s_l[o], w_l[o], tile, bank))
        np.add.at(counts_all[c], (tile, bank), 1)

    # static padded counts per (tile, bank): max over cores, ceil to CHUNK
    P = ((counts_all.max(axis=0) + CHUNK - 1) // CHUNK) * CHUNK  # [T, NBANKS]
    nblk = P // CHUNK

    # segment-iteration order (group, bank, tile) -> slot offsets
    n_groups = (T_TILES + GT - 1) // GT
    seg_order = []  # (t, b) in iteration order
    for g in range(n_groups):
        for b in range(NBANKS):
            for t in range(g * GT, min((g + 1) * GT, T_TILES)):
                seg_order.append((t, b))
    seg_sizes = np.array([P[t, b] for t, b in seg_order], np.int64)
    seg_starts = np.concatenate([[0], np.cumsum(seg_sizes)])
    tot_slots = int(seg_starts[-1])
    totblk = tot_slots // CHUNK
    # map (t, b) -> slot start
    seg_start_tb = np.zeros((T_TILES, NBANKS), np.int64)
    for i, (t, b) in enumerate(seg_order):
        seg_start_tb[t, b] = seg_starts[i]

    # per-group metadata for the program builder
    groups = []
    for g in range(n_groups):
        tiles_g = list(range(g * GT, min((g + 1) * GT, T_TILES)))
        g_slot0 = int(seg_start_tb[tiles_g[0], 0])
        g_blk0 = g_slot0 // CHUNK
        bank_segs = []  # (local block offset, num blocks) per bank
        for b in range(NBANKS):
            s0 = int(seg_start_tb[tiles_g[0], b])
            ln = int(sum(P[t, b] for t in tiles_g))
            bank_segs.append(((s0 - g_slot0) // CHUNK, ln // CHUNK))
        tile_blocks = []  # per tile: list of local block indices (group-rel)
        for t in tiles_g:
            blks = []
            for b in range(NBANKS):
                s0 = int(seg_start_tb[t, b])
                blks.extend(range((s0 - g_slot0) // CHUNK,
                                  (s0 - g_slot0) // CHUNK + int(nblk[t, b])))
            tile_blocks.append((t, blks))
        nblk_g = int(sum(P[t, b] for t in tiles_g for b in range(NBANKS))) // CHUNK
        groups.append({
            "blk0": g_blk0, "nblk": nblk_g,
            "bank_segs": bank_segs, "tile_blocks": tile_blocks,
        })

    # per-core streams
    per_core = []
    for c in range(NCORES):
        d_l, s_l, w_l, tile, bank = cores[c]
        ne = len(d_l)
        # edges are sorted in segment-iteration order; compute positions
        cnt_iter = np.array(
            [counts_all[c, t, b] for t, b in seg_order], np.int64)
        first = np.concatenate([[0], np.cumsum(cnt_iter)])[:-1]
        pos = (np.repeat(seg_starts[:-1], cnt_iter)
               + np.arange(ne) - np.repeat(first, cnt_iter))

        # pad slots repeat the segment's last real src row (HBM row stays
        # hot) and carry w=0 so they contribute nothing.
        idx_stream = np.zeros(tot_slots, np.int16)
        wq_stream = np.zeros(tot_slots, np.float32)
        dl_stream = np.zeros(tot_slots, np.float32)
        idx_stream[pos] = (s_l % BROWS).astype(np.int16)
        # forward-fill pad slots within each segment from the last real edge
        valid = np.zeros(tot_slots, bool)
        valid[pos] = True
        vi = np.where(valid, np.arange(tot_slots), 0)
        np.maximum.accumulate(vi, out=vi)
        idx_stream = idx_stream[vi]
        wq_stream[pos] = w_l
        dl_stream[pos] = (d_l % TILE_W).astype(np.float32)

        idx16 = np.ascontiguousarray(
            np.tile(idx_stream.reshape(-1, 16).T, (8, 1)))   # [128, totblk*8]
        dl_cols = np.ascontiguousarray(
            dl_stream.reshape(totblk, CHUNK).T)               # [128, totblk]
        w_cols = np.ascontiguousarray(
            wq_stream.reshape(totblk, CHUNK).T)               # [128, totblk]
        per_core.append({"idx16": idx16, "dl": dl_cols, "w": w_cols})

    xb = np.asarray(x, np.float32).astype(bf16)  # [N, D]
    banks = [np.ascontiguousarray(xb[b * BROWS:(b + 1) * BROWS])
             for b in range(NBANKS)]

    iota = np.ascontiguousarray(np.broadcast_to(
        np.arange(TILE_W, dtype=np.float32), (128, TILE_W)).astype(bf16))

    return per_core, banks, iota, groups, totblk


def _build_program(groups, totblk):
    import concourse.bass as bass  # noqa: F401
    import concourse.bacc as bacc
    import concourse.mybir as mybir
    import concourse.tile as tile
    from concourse import library_config

    nc = bacc.Bacc("TRN2", target_bir_lowering=False, debug=False,
                   num_devices=NCORES, num_swdge_queues=NQUEUES)

    xb_d = [nc.dram_tensor(f"xb{b}", [BROWS, D], mybir.dt.bfloat16,
                           kind="ExternalInput") for b in range(NBANKS)]
    idx_d = nc.dram_tensor("idx16", [128, totblk * 8], mybir.dt.int16,
                           kind="ExternalInput")
    dl_d = nc.dram_tensor("dl", [128, totblk], mybir.dt.float32,
                          kind="ExternalInput")
    w_d = nc.dram_tensor("w", [128, totblk], mybir.dt.float32,
                         kind="ExternalInput")
    iota_d = nc.dram_tensor("iota", [128, TILE_W], mybir.dt.bfloat16,
                            kind="ExternalInput")
    wt_d = nc.dram_tensor("wt", [D, D], mybir.dt.float32, kind="ExternalInput")
    b_d = nc.dram_tensor("b", [D, 1], mybir.dt.float32, kind="ExternalInput")
    out_d = nc.dram_tensor("outT", [D, T_TILES * TILE_W], mybir.dt.float32,
                           kind="ExternalOutput")

    max_nblk = max(g["nblk"] for g in groups)
    SUP_W = GT * TILE_W  # supertile width (512)

    with tile.TileContext(nc) as tc:
        with (
            tc.tile_pool(name="const", bufs=1) as constp,
            tc.tile_pool(name="meta", bufs=3) as metap,
            tc.tile_pool(name="gather", bufs=3) as gatherp,
            tc.tile_pool(name="s", bufs=6) as sp,
            tc.tile_pool(name="agg", bufs=2) as aggp,
            tc.tile_pool(name="outp", bufs=2) as outp,
            tc.tile_pool(name="psum", bufs=4, space="PSUM") as psump,
            tc.tile_pool(name="psum2", bufs=2, space="PSUM") as psum2p,
        ):
            nc.gpsimd.load_library(library_config.mlp)

            iota_t = constp.tile([128, TILE_W], mybir.dt.bfloat16)
            wt_t = constp.tile([D, D], mybir.dt.float32)
            b_t = constp.tile([D, 1], mybir.dt.float32)
            nc.sync.dma_start(iota_t[:], iota_d[:])
            nc.sync.dma_start(wt_t[:], wt_d[:])
            nc.sync.dma_start(b_t[:], b_d[:])

            for gi, g in enumerate(groups):
                blk0, nblk_g = g["blk0"], g["nblk"]
                ntile_g = len(g["tile_blocks"])
                sup_w = ntile_g * TILE_W
                idx_t = metap.tile([128, max_nblk * 8], mybir.dt.int16,
                                   tag="idx")
                dl_t = metap.tile([128, max_nblk], mybir.dt.float32, tag="dl")
                w_t = metap.tile([128, max_nblk], mybir.dt.float32, tag="w")
                nc.sync.dma_start(idx_t[:, :nblk_g * 8],
                                  idx_d[:, blk0 * 8:(blk0 + nblk_g) * 8])
                nc.sync.dma_start(dl_t[:, :nblk_g],
                                  dl_d[:, blk0:blk0 + nblk_g])
                nc.sync.dma_start(w_t[:, :nblk_g],
                                  w_d[:, blk0:blk0 + nblk_g])

                g_t = gatherp.tile([128, max_nblk, D], mybir.dt.bfloat16,
                                   tag="g")
                # One SWDGE ring holds 1024 descriptors (16 KB carveout /
                # 16 B), so one gather call covers up to 8 blocks of 128
                # rows. Bank b's calls go to SWDGE queue b: descriptor
                # generation for the 4 banks runs on 4 distinct Q7 core
                # pairs concurrently. Calls are issued bank-interleaved so
                # the 8-deep GpSimd engine queue always spans all 4 queues.
                MAXG = 8
                call_lists = []  # per bank: list of (a, ln)
                for b in range(NBANKS):
                    boff, blen = g["bank_segs"][b]
                    calls = []
                    for off in range(0, blen, MAXG):
                        calls.append((boff + off, min(MAXG, blen - off)))
                    call_lists.append(calls)
                for ci in range(max(len(cl) for cl in call_lists)):
                    for b in range(NBANKS):
                        if ci >= len(call_lists[b]):
                            continue
                        a, ln = call_lists[b][ci]
                        nc.gpsimd.dma_gather(
                            g_t[:, a:a + ln, :],
                            xb_d[b][:],
                            idx_t[:, a * 8:(a + ln) * 8],
                            ln * CHUNK,
                            ln * CHUNK,
                            D,
                            queue_num=b,
                        )

                agg4_t = aggp.tile([D, SUP_W], mybir.dt.float32, tag="agg")
                for ti, (t, blks) in enumerate(g["tile_blocks"]):
                    assert blks, f"tile {t} has no edge blocks"
                    psum_t = psump.tile([D, TILE_W], mybir.dt.float32,
                                        tag="p1")
                    for k, j in enumerate(blks):
                        s_t = sp.tile([128, TILE_W], mybir.dt.bfloat16,
                                      tag="s")
                        nc.vector.tensor_scalar(
                            s_t[:], iota_t[:], dl_t[:, j:j + 1],
                            w_t[:, j:j + 1],
                            mybir.AluOpType.is_equal, mybir.AluOpType.mult,
                        )
                        nc.tensor.matmul(
                            psum_t[:], g_t[:, j, :], s_t[:],
                            start=(k == 0), stop=(k == len(blks) - 1),
                        )

                    nc.scalar.copy(
                        agg4_t[:, ti * TILE_W:(ti + 1) * TILE_W], psum_t[:])

                psum2_t = psum2p.tile([D, SUP_W], mybir.dt.float32, tag="p2")
                nc.tensor.matmul(psum2_t[:, :sup_w], wt_t[:],
                                 agg4_t[:, :sup_w], start=True, stop=True)

                out_t = outp.tile([D, SUP_W], mybir.dt.float32, tag="o")
                nc.scalar.activation(
                    out_t[:, :sup_w], psum2_t[:, :sup_w],
                    mybir.ActivationFunctionType.Identity,
                    bias=b_t[:, 0:1], scale=1.0,
                )
                t0 = g["tile_blocks"][0][0]
                nc.sync.dma_start(
                    out_d[:, t0 * TILE_W:t0 * TILE_W + sup_w],
                    out_t[:, :sup_w])

    nc.compile()
    return nc


LAST_RES = None


def kernel(x, edge_index, edge_weight, W, b):
    import os
    from concourse.bass_utils import run_bass_kernel_spmd

    per_core, banks, iota, groups, totblk = _host_prep(
        x, edge_index, edge_weight)

    nc = _build_program(groups, totblk)

    WT = np.ascontiguousarray(np.asarray(W, np.float32).T)  # [din, dout]
    bcol = np.ascontiguousarray(np.asarray(b, np.float32).reshape(D, 1))

    in_maps = []
    for c in range(NCORES):
        p = per_core[c]
        m = {f"xb{i}": banks[i] for i in range(NBANKS)}
        m.update({
            "idx16": p["idx16"], "dl": p["dl"], "w": p["w"],
            "iota": iota, "wt": WT, "b": bcol,
        })
        in_maps.append(m)

    res = run_bass_kernel_spmd(
        nc, in_maps, core_ids=list(range(NCORES)),
        trace=bool(int(os.environ.get("KERNEL_TRACE", "0"))),
    )
    global LAST_RES
    LAST_RES = res

    out = np.empty((N_NODES, D), np.float32)
    for c in range(NCORES):
        outT = res.results[c]["outT"]  # [D, T*TILE_W]
        out[c * NLOC:(c + 1) * NLOC] = outT[:, :NLOC].T
    return out


if __name__ == "__main__":
    # smoke test with random data (self-contained)
    rng = np.random.default_rng(0)
    x = rng.standard_normal((N_NODES, D)).astype(np.float32)
    ei = rng.integers(0, N_NODES, size=(2, N_EDGES)).astype(np.int64)
    ew = rng.random(N_EDGES).astype(np.float32)
    W = (rng.standard_normal((D, D)) / np.sqrt(D)).astype(np.float32)
    b = (rng.standard_normal(D) * 0.01).astype(np.float32)
    out = kernel(x, ei, ew, W, b)
    print("out", out.shape, out.dtype)


# revision 7
# speedup vs baseline: 2.9535x; 1.3733x over previous
"""DirGCNConv (weighted gather + segment_sum + linear) on 8 Trainium2 NeuronCores.

v4: like v3 (4 SWDGE gather queues, dst tiles of 128), but the weighted
one-hot scatter matrices S are precomputed on the host and streamed from HBM
instead of being built per chunk on the DVE. This removes the per-chunk
tensor_scalar (was ~300 ns x 3.5k = 1.05 ms of DVE time) at the cost of
~113 MB/core of extra sequential HBM reads, and lets the chunk matmuls
pipeline back-to-back on TensorE.

  - Host: shard edges by dst range (12500/core); dst tiles of 128; edges
    grouped by (gather group of GT=3 tiles, src bank of 25000, tile), sorted
    by src, padded per (tile, bank) to 128-edge chunks (max over cores, one
    SPMD program). Streams: int16 bank-local gather indices (idx16), dense
    S[slot, 128] bf16 with S[slot, dl_slot] = w_slot (zeros on pad slots).
  - Device, per group: dma_gather x rows on SWDGE queue=bank (4 Q7 pairs in
    parallel); DMA the group's S blocks; per chunk j: TensorE PSUM
    agg.T[din, dst128] += G_j.T @ S_j; per tile: ACT copy PSUM -> agg
    staging [128, 384]; per group: out.T = W @ agg + b (matmul + ACT bias),
    one DMA out.
  - Host: out rows of core c = outT[:, :12500].T.
"""

import numpy as np
import ml_dtypes

bf16 = ml_dtypes.bfloat16

# problem constants (hardcoded per harness contract)
N_NODES = 100000
N_EDGES = 3200000
D = 128
NCORES = 8

# design constants
NLOC = N_NODES // NCORES      # 12500 dst nodes per core
TILE_W = 128                  # dst columns per tile
T_TILES = (NLOC + TILE_W - 1) // TILE_W   # 98
CHUNK = 128                   # edges per matmul chunk
NBANKS = 4
BROWS = N_NODES // NBANKS     # 25000 rows per src bank (int16-safe)
GT = 3                        # tiles per gather group / output supertile
NQUEUES = 4                   # SWDGE descriptor queues (Q7 core pairs)


def _host_prep(x, edge_index, edge_weight):
    """Vectorized host prep. Returns per-core streams + static block counts."""
    dst = np.asarray(edge_index[0], dtype=np.int64)
    src = np.asarray(edge_index[1], dtype=np.int64)
    w = np.asarray(edge_weight, dtype=np.float32)

    order = np.argsort(dst, kind="stable")
    dst_s, src_s, w_s = dst[order], src[order], w[order]
    bounds = np.searchsorted(dst_s, np.arange(NCORES + 1) * NLOC)

    cores = []
    counts_all = np.zeros((NCORES, T_TILES, NBANKS), np.int64)
    for c in range(NCORES):
        lo, hi = int(bounds[c]), int(bounds[c + 1])
        d_l = dst_s[lo:hi] - c * NLOC
        s_l = src_s[lo:hi]
        w_l = w_s[lo:hi]
        tile = d_l // TILE_W
        bank = s_l // BROWS
        grp = tile // GT
        # sort into segment-iteration order: (group, bank, tile), src minor
        o = np.lexsort((s_l, tile, bank, grp))
        tile, bank = tile[o], bank[o]
        cores.append((d_l[o], s_l[o], w_l[o], tile, bank))
        np.add.at(counts_all[c], (tile, bank), 1)

    # static padded counts per (tile, bank): max over cores, ceil to CHUNK
    P = ((counts_all.max(axis=0) + CHUNK - 1) // CHUNK) * CHUNK  # [T, NBANKS]
    nblk = P // CHUNK

    # segment-iteration order (group, bank, tile) -> slot offsets
    n_groups = (T_TILES + GT - 1) // GT
    seg_order = []  # (t, b) in iteration order
    for g in range(n_groups):
        for b in range(NBANKS):
            for t in range(g * GT, min((g + 1) * GT, T_TILES)):
                seg_order.append((t, b))
    seg_sizes = np.array([P[t, b] for t, b in seg_order], np.int64)
    seg_starts = np.concatenate([[0], np.cumsum(seg_sizes)])
    tot_slots = int(seg_starts[-1])
    totblk = tot_slots // CHUNK
    # map (t, b) -> slot start
    seg_start_tb = np.zeros((T_TILES, NBANKS), np.int64)
    for i, (t, b) in enumerate(seg_order):
        seg_start_tb[t, b] = seg_starts[i]

    # per-group metadata for the program builder
    groups = []
    for g in range(n_groups):
        tiles_g = list(range(g * GT, min((g + 1) * GT, T_TILES)))
        g_slot0 = int(seg_start_tb[tiles_g[0], 0])
        g_blk0 = g_slot0 // CHUNK
        bank_segs = []  # (local block offset, num blocks) per bank
        for b in range(NBANKS):
            s0 = int(seg_start_tb[tiles_g[0], b])
            ln = int(sum(P[t, b] for t in tiles_g))
            bank_segs.append(((s0 - g_slot0) // CHUNK, ln // CHUNK))
        tile_blocks = []  # per tile: list of local block indices (group-rel)
        for t in tiles_g:
            blks = []
            for b in range(NBANKS):
                s0 = int(seg_start_tb[t, b])
                blks.extend(range((s0 - g_slot0) // CHUNK,
                                  (s0 - g_slot0) // CHUNK + int(nblk[t, b])))
            tile_blocks.append((t, blks))
        nblk_g = int(sum(P[t, b] for t in tiles_g for b in range(NBANKS))) // CHUNK
        groups.append({
            "blk0": g_blk0, "nblk": nblk_g,
            "bank_segs": bank_segs, "tile_blocks": tile_blocks,
        })

    # per-core streams
    per_core = []
    for c in range(NCORES):
        d_l, s_l, w_l, tile, bank = cores[c]
        ne = len(d_l)
        # edges are sorted in segment-iteration order; compute positions
        cnt_iter = np.array(
            [counts_all[c, t, b] for t, b in seg_order], np.int64)
        first = np.concatenate([[0], np.cumsum(cnt_iter)])[:-1]
        pos = (np.repeat(seg_starts[:-1], cnt_iter)
               + np.arange(ne) - np.repeat(first, cnt_iter))

        # pad slots repeat the segment's last real src row (HBM row stays
        # hot); their S row is all-zero so they contribute nothing.
        idx_stream = np.zeros(tot_slots, np.int16)
        idx_stream[pos] = (s_l % BROWS).astype(np.int16)
        valid = np.zeros(tot_slots, bool)
        valid[pos] = True
        vi = np.where(valid, np.arange(tot_slots), 0)
        np.maximum.accumulate(vi, out=vi)
        idx_stream = idx_stream[vi]

        idx16 = np.ascontiguousarray(
            np.tile(idx_stream.reshape(-1, 16).T, (8, 1)))   # [128, totblk*8]

        # dense weighted one-hot stream: S[slot, dl_slot] = w_slot
        s_flat = np.zeros((tot_slots, TILE_W), np.float32)
        s_flat[pos, (d_l % TILE_W)] = w_l
        s_np = np.ascontiguousarray(
            s_flat.astype(bf16).reshape(totblk, CHUNK, TILE_W)
            .transpose(1, 0, 2))                             # [128, totblk, T]
        per_core.append({"idx16": idx16, "s": s_np})

    xb = np.asarray(x, np.float32).astype(bf16)  # [N, D]
    banks = [np.ascontiguousarray(xb[b * BROWS:(b + 1) * BROWS])
             for b in range(NBANKS)]

    return per_core, banks, groups, totblk


def _build_program(groups, totblk):
    import concourse.bass as bass  # noqa: F401
    import concourse.bacc as bacc
    import concourse.mybir as mybir
    import concourse.tile as tile
    from concourse import library_config

    nc = bacc.Bacc("TRN2", target_bir_lowering=False, debug=False,
                   num_devices=NCORES, num_swdge_queues=NQUEUES)

    xb_d = [nc.dram_tensor(f"xb{b}", [BROWS, D], mybir.dt.bfloat16,
                           kind="ExternalInput") for b in range(NBANKS)]
    idx_d = nc.dram_tensor("idx16", [128, totblk * 8], mybir.dt.int16,
                           kind="ExternalInput")
    s_d = nc.dram_tensor("s", [128, totblk, TILE_W], mybir.dt.bfloat16,
                         kind="ExternalInput")
    wt_d = nc.dram_tensor("wt", [D, D], mybir.dt.float32, kind="ExternalInput")
    b_d = nc.dram_tensor("b", [D, 1], mybir.dt.float32, kind="ExternalInput")
    out_d = nc.dram_tensor("outT", [D, T_TILES * TILE_W], mybir.dt.float32,
                           kind="ExternalOutput")

    max_nblk = max(g["nblk"] for g in groups)
    SUP_W = GT * TILE_W  # supertile width (384)

    with tile.TileContext(nc) as tc:
        with (
            tc.tile_pool(name="const", bufs=1) as constp,
            tc.tile_pool(name="meta", bufs=3) as metap,
            tc.tile_pool(name="gather", bufs=2) as gatherp,
            tc.tile_pool(name="s", bufs=2) as sp,
            tc.tile_pool(name="agg", bufs=2) as aggp,
            tc.tile_pool(name="outp", bufs=2) as outp,
            tc.tile_pool(name="psum", bufs=4, space="PSUM") as psump,
            tc.tile_pool(name="psum2", bufs=2, space="PSUM") as psum2p,
        ):
            nc.gpsimd.load_library(library_config.mlp)

            wt_t = constp.tile([D, D], mybir.dt.float32)
            b_t = constp.tile([D, 1], mybir.dt.float32)
            nc.sync.dma_start(wt_t[:], wt_d[:])
            nc.sync.dma_start(b_t[:], b_d[:])

            for gi, g in enumerate(groups):
                blk0, nblk_g = g["blk0"], g["nblk"]
                ntile_g = len(g["tile_blocks"])
                sup_w = ntile_g * TILE_W
                idx_t = metap.tile([128, max_nblk * 8], mybir.dt.int16,
                                   tag="idx")
                nc.sync.dma_start(idx_t[:, :nblk_g * 8],
                                  idx_d[:, blk0 * 8:(blk0 + nblk_g) * 8])

                s_t = sp.tile([128, max_nblk, TILE_W], mybir.dt.bfloat16,
                              tag="s")
                nc.sync.dma_start(s_t[:, :nblk_g, :],
                                  s_d[:, blk0:blk0 + nblk_g, :])

                g_t = gatherp.tile([128, max_nblk, D], mybir.dt.bfloat16,
                                   tag="g")
                # One SWDGE ring holds 1024 descriptors, so one gather call
                # covers up to 8 blocks of 128 rows. Bank b's calls go to
                # SWDGE queue b (4 Q7 core pairs in parallel), issued
                # bank-interleaved so the 8-deep GpSimd engine queue always
                # spans all 4 queues.
                MAXG = 8
                call_lists = []  # per bank: list of (a, ln)
                for b in range(NBANKS):
                    boff, blen = g["bank_segs"][b]
                    calls = []
                    for off in range(0, blen, MAXG):
                        calls.append((boff + off, min(MAXG, blen - off)))
                    call_lists.append(calls)
                for ci in range(max(len(cl) for cl in call_lists)):
                    for b in range(NBANKS):
                        if ci >= len(call_lists[b]):
                            continue
                        a, ln = call_lists[b][ci]
                        nc.gpsimd.dma_gather(
                            g_t[:, a:a + ln, :],
                            xb_d[b][:],
                            idx_t[:, a * 8:(a + ln) * 8],
                            ln * CHUNK,
                            ln * CHUNK,
                            D,
                            queue_num=b,
                        )

                agg4_t = aggp.tile([D, SUP_W], mybir.dt.float32, tag="agg")
                for ti, (t, blks) in enumerate(g["tile_blocks"]):
                    assert blks, f"tile {t} has no edge blocks"
                    psum_t = psump.tile([D, TILE_W], mybir.dt.float32,
                                        tag="p1")
                    for k, j in enumerate(blks):
                        nc.tensor.matmul(
                            psum_t[:], g_t[:, j, :], s_t[:, j, :],
                            start=(k == 0), stop=(k == len(blks) - 1),
                        )

                    nc.scalar.copy(
                        agg4_t[:, ti * TILE_W:(ti + 1) * TILE_W], psum_t[:])

                psum2_t = psum2p.tile([D, SUP_W], mybir.dt.float32, tag="p2")
                nc.tensor.matmul(psum2_t[:, :sup_w], wt_t[:],
                                 agg4_t[:, :sup_w], start=True, stop=True)

                out_t = outp.tile([D, SUP_W], mybir.dt.float32, tag="o")
                nc.scalar.activation(
                    out_t[:, :sup_w], psum2_t[:, :sup_w],
                    mybir.ActivationFunctionType.Identity,
                    bias=b_t[:, 0:1], scale=1.0,
                )
                t0 = g["tile_blocks"][0][0]
                nc.sync.dma_start(
                    out_d[:, t0 * TILE_W:t0 * TILE_W + sup_w],
                    out_t[:, :sup_w])

    nc.compile()
    return nc


LAST_RES = None


def kernel(x, edge_index, edge_weight, W, b):
    import os
    from concourse.bass_utils import run_bass_kernel_spmd

    per_core, banks, groups, totblk = _host_prep(x, edge_index, edge_weight)

    nc = _build_program(groups, totblk)

    WT = np.ascontiguousarray(np.asarray(W, np.float32).T)  # [din, dout]
    bcol = np.ascontiguousarray(np.asarray(b, np.float32).reshape(D, 1))

    in_maps = []
    for c in range(NCORES):
        p = per_core[c]
        m = {f"xb{i}": banks[i] for i in range(NBANKS)}
        m.update({
            "idx16": p["idx16"], "s": p["s"], "wt": WT, "b": bcol,
        })
        in_maps.append(m)

    res = run_bass_kernel_spmd(
        nc, in_maps, core_ids=list(range(NCORES)),
        trace=bool(int(os.environ.get("KERNEL_TRACE", "0"))),
    )
    global LAST_RES
    LAST_RES = res

    out = np.empty((N_NODES, D), np.float32)
    for c in range(NCORES):
        outT = res.results[c]["outT"]  # [D, T*TILE_W]
        out[c * NLOC:(c + 1) * NLOC] = outT[:, :NLOC].T
    return out


if __name__ == "__main__":
    # smoke test with random data (self-contained)
    rng = np.random.default_rng(0)
    x = rng.standard_normal((N_NODES, D)).astype(np.float32)
    ei = rng.integers(0, N_NODES, size=(2, N_EDGES)).astype(np.int64)
    ew = rng.random(N_EDGES).astype(np.float32)
    W = (rng.standard_normal((D, D)) / np.sqrt(D)).astype(np.float32)
    b = (rng.standard_normal(D) * 0.01).astype(np.float32)
    out = kernel(x, ei, ew, W, b)
    print("out", out.shape, out.dtype)


# revision 9
# speedup vs baseline: 3.0141x; 1.0205x over previous
"""DirGCNConv (weighted gather + segment_sum + linear) on 8 Trainium2 NeuronCores.

v4: like v3 (4 SWDGE gather queues, dst tiles of 128), but the weighted
one-hot scatter matrices S are precomputed on the host and streamed from HBM
instead of being built per chunk on the DVE. This removes the per-chunk
tensor_scalar (was ~300 ns x 3.5k = 1.05 ms of DVE time) at the cost of
~113 MB/core of extra sequential HBM reads, and lets the chunk matmuls
pipeline back-to-back on TensorE.

  - Host: shard edges by dst range (12500/core); dst tiles of 128; edges
    grouped by (gather group of GT=3 tiles, src bank of 25000, tile), sorted
    by src, padded per (tile, bank) to 128-edge chunks (max over cores, one
    SPMD program). Streams: int16 bank-local gather indices (idx16), dense
    S[slot, 128] bf16 with S[slot, dl_slot] = w_slot (zeros on pad slots).
  - Device, per group: dma_gather x rows on SWDGE queue=bank (4 Q7 pairs in
    parallel); DMA the group's S blocks; per chunk j: TensorE PSUM
    agg.T[din, dst128] += G_j.T @ S_j; per tile: ACT copy PSUM -> agg
    staging [128, 384]; per group: out.T = W @ agg + b (matmul + ACT bias),
    one DMA out.
  - Host: out rows of core c = outT[:, :12500].T.
"""

import numpy as np
import ml_dtypes

bf16 = ml_dtypes.bfloat16

# problem constants (hardcoded per harness contract)
N_NODES = 100000
N_EDGES = 3200000
D = 128
NCORES = 8

# design constants
NLOC = N_NODES // NCORES      # 12500 dst nodes per core
TILE_W = 128                  # dst columns per tile
T_TILES = (NLOC + TILE_W - 1) // TILE_W   # 98
CHUNK = 128                   # edges per matmul chunk
NBANKS = 4
BROWS = N_NODES // NBANKS     # 25000 rows per src bank (int16-safe)
GT = 3                        # tiles per gather group / output supertile
NQUEUES = 4                   # SWDGE descriptor queues (Q7 core pairs)


def _host_prep(x, edge_index, edge_weight):
    """Vectorized host prep. Returns per-core streams + static block counts."""
    dst = np.asarray(edge_index[0], dtype=np.int64)
    src = np.asarray(edge_index[1], dtype=np.int64)
    w = np.asarray(edge_weight, dtype=np.float32)

    order = np.argsort(dst, kind="stable")
    dst_s, src_s, w_s = dst[order], src[order], w[order]
    bounds = np.searchsorted(dst_s, np.arange(NCORES + 1) * NLOC)

    cores = []
    counts_all = np.zeros((NCORES, T_TILES, NBANKS), np.int64)
    for c in range(NCORES):
        lo, hi = int(bounds[c]), int(bounds[c + 1])
        d_l = dst_s[lo:hi] - c * NLOC
        s_l = src_s[lo:hi]
        w_l = w_s[lo:hi]
        tile = d_l // TILE_W
        bank = s_l // BROWS
        grp = tile // GT
        # sort into segment-iteration order: (group, bank, tile), src minor
        o = np.lexsort((s_l, tile, bank, grp))
        tile, bank = tile[o], bank[o]
        cores.append((d_l[o], s_l[o], w_l[o], tile, bank))
        np.add.at(counts_all[c], (tile, bank), 1)

    # static padded counts per (tile, bank): max over cores, ceil to CHUNK
    P = ((counts_all.max(axis=0) + CHUNK - 1) // CHUNK) * CHUNK  # [T, NBANKS]
    nblk = P // CHUNK

    # segment-iteration order (group, bank, tile) -> slot offsets
    n_groups = (T_TILES + GT - 1) // GT
    seg_order = []  # (t, b) in iteration order
    for g in range(n_groups):
        for b in range(NBANKS):
            for t in range(g * GT, min((g + 1) * GT, T_TILES)):
                seg_order.append((t, b))
    seg_sizes = np.array([P[t, b] for t, b in seg_order], np.int64)
    seg_starts = np.concatenate([[0], np.cumsum(seg_sizes)])
    tot_slots = int(seg_starts[-1])
    totblk = tot_slots // CHUNK
    # map (t, b) -> slot start
    seg_start_tb = np.zeros((T_TILES, NBANKS), np.int64)
    for i, (t, b) in enumerate(seg_order):
        seg_start_tb[t, b] = seg_starts[i]

    # per-group metadata for the program builder
    groups = []
    for g in range(n_groups):
        tiles_g = list(range(g * GT, min((g + 1) * GT, T_TILES)))
        g_slot0 = int(seg_start_tb[tiles_g[0], 0])
        g_blk0 = g_slot0 // CHUNK
        bank_segs = []  # (local block offset, num blocks) per bank
        for b in range(NBANKS):
            s0 = int(seg_start_tb[tiles_g[0], b])
            ln = int(sum(P[t, b] for t in tiles_g))
            bank_segs.append(((s0 - g_slot0) // CHUNK, ln // CHUNK))
        tile_blocks = []  # per tile: list of local block indices (group-rel)
        for t in tiles_g:
            blks = []
            for b in range(NBANKS):
                s0 = int(seg_start_tb[t, b])
                blks.extend(range((s0 - g_slot0) // CHUNK,
                                  (s0 - g_slot0) // CHUNK + int(nblk[t, b])))
            tile_blocks.append((t, blks))
        nblk_g = int(sum(P[t, b] for t in tiles_g for b in range(NBANKS))) // CHUNK
        groups.append({
            "blk0": g_blk0, "nblk": nblk_g,
            "bank_segs": bank_segs, "tile_blocks": tile_blocks,
        })

    # per-core streams
    per_core = []
    for c in range(NCORES):
        d_l, s_l, w_l, tile, bank = cores[c]
        ne = len(d_l)
        # edges are sorted in segment-iteration order; compute positions
        cnt_iter = np.array(
            [counts_all[c, t, b] for t, b in seg_order], np.int64)
        first = np.concatenate([[0], np.cumsum(cnt_iter)])[:-1]
        pos = (np.repeat(seg_starts[:-1], cnt_iter)
               + np.arange(ne) - np.repeat(first, cnt_iter))

        # pad slots repeat the segment's last real src row (HBM row stays
        # hot); their S row is all-zero so they contribute nothing.
        idx_stream = np.zeros(tot_slots, np.int16)
        idx_stream[pos] = (s_l % BROWS).astype(np.int16)
        valid = np.zeros(tot_slots, bool)
        valid[pos] = True
        vi = np.where(valid, np.arange(tot_slots), 0)
        np.maximum.accumulate(vi, out=vi)
        idx_stream = idx_stream[vi]

        idx16 = np.ascontiguousarray(
            np.tile(idx_stream.reshape(-1, 16).T, (8, 1)))   # [128, totblk*8]

        # dense weighted one-hot stream: S[slot, dl_slot] = w_slot
        s_flat = np.zeros((tot_slots, TILE_W), np.float32)
        s_flat[pos, (d_l % TILE_W)] = w_l
        s_np = np.ascontiguousarray(
            s_flat.astype(bf16).reshape(totblk, CHUNK, TILE_W)
            .transpose(1, 0, 2))                             # [128, totblk, T]
        per_core.append({"idx16": idx16, "s": s_np})

    xb = np.asarray(x, np.float32).astype(bf16)  # [N, D]
    banks = [np.ascontiguousarray(xb[b * BROWS:(b + 1) * BROWS])
             for b in range(NBANKS)]

    return per_core, banks, groups, totblk


def _build_program(groups, totblk):
    import concourse.bass as bass  # noqa: F401
    import concourse.bacc as bacc
    import concourse.mybir as mybir
    import concourse.tile as tile
    from concourse import library_config

    nc = bacc.Bacc("TRN2", target_bir_lowering=False, debug=False,
                   num_devices=NCORES, num_swdge_queues=NQUEUES)

    xb_d = [nc.dram_tensor(f"xb{b}", [BROWS, D], mybir.dt.bfloat16,
                           kind="ExternalInput") for b in range(NBANKS)]
    idx_d = nc.dram_tensor("idx16", [128, totblk * 8], mybir.dt.int16,
                           kind="ExternalInput")
    s_d = nc.dram_tensor("s", [128, totblk, TILE_W], mybir.dt.bfloat16,
                         kind="ExternalInput")
    wt_d = nc.dram_tensor("wt", [D, D], mybir.dt.float32, kind="ExternalInput")
    b_d = nc.dram_tensor("b", [D, 1], mybir.dt.float32, kind="ExternalInput")
    out_d = nc.dram_tensor("outT", [D, T_TILES * TILE_W], mybir.dt.float32,
                           kind="ExternalOutput")

    max_nblk = max(g["nblk"] for g in groups)
    SUP_W = GT * TILE_W  # supertile width (384)

    with tile.TileContext(nc) as tc:
        with (
            tc.tile_pool(name="const", bufs=1) as constp,
            tc.tile_pool(name="meta", bufs=3) as metap,
            tc.tile_pool(name="gather", bufs=3) as gatherp,
            tc.tile_pool(name="s", bufs=2) as sp,
            tc.tile_pool(name="agg", bufs=2) as aggp,
            tc.tile_pool(name="outp", bufs=2) as outp,
            tc.tile_pool(name="psum", bufs=4, space="PSUM") as psump,
            tc.tile_pool(name="psum2", bufs=2, space="PSUM") as psum2p,
        ):
            nc.gpsimd.load_library(library_config.mlp)

            wt_t = constp.tile([D, D], mybir.dt.float32)
            b_t = constp.tile([D, 1], mybir.dt.float32)
            nc.sync.dma_start(wt_t[:], wt_d[:])
            nc.sync.dma_start(b_t[:], b_d[:])

            for gi, g in enumerate(groups):
                blk0, nblk_g = g["blk0"], g["nblk"]
                ntile_g = len(g["tile_blocks"])
                sup_w = ntile_g * TILE_W
                idx_t = metap.tile([128, max_nblk * 8], mybir.dt.int16,
                                   tag="idx")
                nc.sync.dma_start(idx_t[:, :nblk_g * 8],
                                  idx_d[:, blk0 * 8:(blk0 + nblk_g) * 8])

                s_t = sp.tile([128, max_nblk, TILE_W], mybir.dt.bfloat16,
                              tag="s")
                nc.sync.dma_start(s_t[:, :nblk_g, :],
                                  s_d[:, blk0:blk0 + nblk_g, :])

                g_t = gatherp.tile([128, max_nblk, D], mybir.dt.bfloat16,
                                   tag="g")
                # One SWDGE ring holds 1024 descriptors, so one gather call
                # covers up to 8 blocks of 128 rows. Bank b's calls go to
                # SWDGE queue b (4 Q7 core pairs in parallel), issued
                # bank-interleaved so the 8-deep GpSimd engine queue always
                # spans all 4 queues. GpSimd retires in order, so each
                # 4-queue "wave" advances at the pace of its largest call:
                # split every bank into the same number of near-equal calls
                # so no wave is ragged.
                MAXG = 8
                ncalls = max((g["bank_segs"][b][1] + MAXG - 1) // MAXG
                             for b in range(NBANKS))
                call_lists = []  # per bank: list of (a, ln)
                for b in range(NBANKS):
                    boff, blen = g["bank_segs"][b]
                    base, rem = divmod(blen, ncalls)
                    calls = []
                    a = boff
                    for ci in range(ncalls):
                        ln = base + (1 if ci < rem else 0)
                        if ln:
                            calls.append((a, ln))
                            a += ln
                    call_lists.append(calls)
                for ci in range(ncalls):
                    for b in range(NBANKS):
                        if ci >= len(call_lists[b]):
                            continue
                        a, ln = call_lists[b][ci]
                        nc.gpsimd.dma_gather(
                            g_t[:, a:a + ln, :],
                            xb_d[b][:],
                            idx_t[:, a * 8:(a + ln) * 8],
                            ln * CHUNK,
                            ln * CHUNK,
                            D,
                            queue_num=b,
                        )

                agg4_t = aggp.tile([D, SUP_W], mybir.dt.float32, tag="agg")
                for ti, (t, blks) in enumerate(g["tile_blocks"]):
                    assert blks, f"tile {t} has no edge blocks"
                    psum_t = psump.tile([D, TILE_W], mybir.dt.float32,
                                        tag="p1")
                    for k, j in enumerate(blks):
                        nc.tensor.matmul(
                            psum_t[:], g_t[:, j, :], s_t[:, j, :],
                            start=(k == 0), stop=(k == len(blks) - 1),
                        )

                    nc.scalar.copy(
                        agg4_t[:, ti * TILE_W:(ti + 1) * TILE_W], psum_t[:])

                psum2_t = psum2p.tile([D, SUP_W], mybir.dt.float32, tag="p2")
                nc.tensor.matmul(psum2_t[:, :sup_w], wt_t[:],
                                 agg4_t[:, :sup_w], start=True, stop=True)

                out_t = outp.tile([D, SUP_W], mybir.dt.float32, tag="o")
                nc.scalar.activation(
                    out_t[:, :sup_w], psum2_t[:, :sup_w],
                    mybir.ActivationFunctionType.Identity,
                    bias=b_t[:, 0:1], scale=1.0,
                )
                t0 = g["tile_blocks"][0][0]
                nc.sync.dma_start(
                    out_d[:, t0 * TILE_W:t0 * TILE_W + sup_w],
                    out_t[:, :sup_w])

    nc.compile()
    return nc


LAST_RES = None


def kernel(x, edge_index, edge_weight, W, b):
    import os
    from concourse.bass_utils import run_bass_kernel_spmd

    per_core, banks, groups, totblk = _host_prep(x, edge_index, edge_weight)

    nc = _build_program(groups, totblk)

    WT = np.ascontiguousarray(np.asarray(W, np.float32).T)  # [din, dout]
    bcol = np.ascontiguousarray(np.asarray(b, np.float32).reshape(D, 1))

    in_maps = []
    for c in range(NCORES):
        p = per_core[c]
        m = {f"xb{i}": banks[i] for i in range(NBANKS)}
        m.update({
            "idx16": p["idx16"], "s": p["s"], "wt": WT, "b": bcol,
        })
        in_maps.append(m)

    res = run_bass_kernel_spmd(
        nc, in_maps, core_ids=list(range(NCORES)),
        trace=bool(int(os.environ.get("KERNEL_TRACE", "0"))),
    )
    global LAST_RES
    LAST_RES = res

    out = np.empty((N_NODES, D), np.float32)
    for c in range(NCORES):
        outT = res.results[c]["outT"]  # [D, T*TILE_W]
        out[c * NLOC:(c + 1) * NLOC] = outT[:, :NLOC].T
    return out


if __name__ == "__main__":
    # smoke test with random data (self-contained)
    rng = np.random.default_rng(0)
    x = rng.standard_normal((N_NODES, D)).astype(np.float32)
    ei = rng.integers(0, N_NODES, size=(2, N_EDGES)).astype(np.int64)
    ew = rng.random(N_EDGES).astype(np.float32)
    W = (rng.standard_normal((D, D)) / np.sqrt(D)).astype(np.float32)
    b = (rng.standard_normal(D) * 0.01).astype(np.float32)
    out = kernel(x, ei, ew, W, b)
    print("out", out.shape, out.dtype)
